# revision 10
# baseline (speedup 1.0000x reference)
"""AGNN (2-layer) distributed Bass kernel for one TRN2 chip (8 NeuronCores).

Strategy (dst-sharded graph parallel):
  - Nodes padded to NPAD = NBLK*128; core c owns BPC consecutive blocks.
  - Per layer a node table [NPAD, 128ch bf16] lives in DRAM: ch 0..63 =
    l2-normalized features nh, ch 64 = max(||h||, 1e-12), ch 65..127 = 0.
  - Edges sorted by (dst block, src half, src). Each (block, half) segment
    padded to CAP chunks of 128 edges. Src rows fetched with dma_gather
    (ascending idx within a segment => coalesced descriptors); dst rows
    from a private per-core copy of its own rows.
  - Per chunk: cos via row-dot (DVE), ee = exp(beta*cos) (ScalarE), then
    an indicator matmul [U|den] += St.T @ [ee*norm_src*nh_src | ee]
    accumulated in PSUM per block (TensorE).
    Epilogue: h' = relu(U / max(den, tiny)).
  - One AllGather distributes each core's new rows between layers.

kernel(**inputs) takes FULL inputs, returns the FULL [50000, 64] output.
Graph preprocessing (sort/pad/index packing) is host numpy; all feature
compute runs on the 8 NeuronCores in a single NEFF.
"""
import contextlib
import numpy as np
import ml_dtypes

import concourse.bass as bass
import concourse.tile as tile
from concourse import bacc, mybir
from concourse.bass_utils import run_bass_kernel_spmd

BF16 = ml_dtypes.bfloat16

# ---------------- geometry (defaults = the real problem) ----------------
N_NODES = 50000
D = 64
N_LAYERS = 2
N_CORES = 8
P = 128

NBLK = 392                 # node blocks of 128
CAP = 9                    # chunks of 128 edges per (block, half) segment
TB = 3                     # blocks per gather tile
TBN = 28                   # blocks per normalize tile (must divide NBLK)

NPAD = NBLK * P
BPC = NBLK // N_CORES
ROWS_PC = BPC * P
H = NPAD // 2
CH = 128
W_CH = 72

_EXEC = {}
_SKIP = set()      # timing-attribution: 'st','dve','gather','mm','epi'
_GATHERS_ONLY = False
_SINGLE = False    # build single-core (TimelineSim) variant


def _set_geometry(n_nodes, nblk, cap, tb, tbn):
    global N_NODES, NBLK, CAP, TB, TBN, NPAD, BPC, ROWS_PC, H
    N_NODES, NBLK, CAP, TB, TBN = n_nodes, nblk, cap, tb, tbn
    NPAD = NBLK * P
    BPC = NBLK // N_CORES
    ROWS_PC = BPC * P
    H = NPAD // 2
    _EXEC.clear()


# ---------------- host-side graph preprocessing ----------------
def _wrap_idx(idx):
    """int16 [n] -> [128, n//16]: position i -> (partition i%16, col i//16),
    replicated across the 8 Q7 core groups."""
    n = len(idx)
    w = idx.reshape(n // 16, 16).T
    iw = np.empty((P, n // 16), np.int16)
    for rep in range(8):
        iw[rep * 16:(rep + 1) * 16] = w
    return np.ascontiguousarray(iw)


def _preprocess(src, dst):
    src = np.asarray(src, np.int64)
    dst = np.asarray(dst, np.int64)
    blk = dst // P
    half = (src >= H).astype(np.int64)
    order = (np.lexsort((src, half, blk)) if _SORT_BY == 'src'
             else np.lexsort((dst, half, blk)))
    s, d, b, h = src[order], dst[order], blk[order], half[order]

    seg = b * 2 + h
    seg_start = np.searchsorted(seg, np.arange(2 * NBLK))
    seg_end = np.searchsorted(seg, np.arange(2 * NBLK), side="right")

    per_core = []
    ns = BPC * CAP * P
    for c in range(N_CORES):
        b0 = c * BPC
        dbase = c * ROWS_PC
        sidx = [np.zeros(ns, np.int16), np.zeros(ns, np.int16)]
        didx = np.zeros(2 * ns, np.int16)
        dloc = np.full(2 * ns, -1.0, np.float32)
        for bl in range(BPC):
            for hh in (0, 1):
                g = (b0 + bl) * 2 + hh
                lo, hi = seg_start[g], seg_end[g]
                k = hi - lo
                assert k <= CAP * P, f"segment {g}: {k} edges > CAP*128"
                if k == 0:
                    continue
                ss, dd = s[lo:hi], d[lo:hi]
                sb = bl * CAP * P
                db = (bl * 2 + hh) * CAP * P
                v = (ss - hh * H).astype(np.int16)
                sidx[hh][sb:sb + k] = v
                sidx[hh][sb + k:sb + CAP * P] = v[-1]
                vd = (dd - dbase).astype(np.int16)
                didx[db:db + k] = vd
                didx[db + k:db + CAP * P] = vd[-1]
                dloc[db:db + k] = (dd - (b0 + bl) * P).astype(np.float32)
        dloct = np.ascontiguousarray(
            dloc.reshape(2 * BPC * CAP, P).T.astype(BF16))
        per_core.append({
            "sidx0": _wrap_idx(sidx[0]),
            "sidx1": _wrap_idx(sidx[1]),
            "didx": _wrap_idx(didx),
            "dloct": dloct,
        })
    return per_core


# ---------------- device kernel builder ----------------
def _build():
    nc = bacc.Bacc("TRN2", target_bir_lowering=False, debug=False,
                   num_devices=N_CORES)
    f32, bf16, i16, i32 = (mybir.dt.float32, mybir.dt.bfloat16,
                           mybir.dt.int16, mybir.dt.int32)

    xt_ext = nc.dram_tensor("xt", [P, NBLK * D], f32, kind="ExternalInput")
    s0_ext = nc.dram_tensor("sidx0", [P, BPC * CAP * 8], i16, kind="ExternalInput")
    s1_ext = nc.dram_tensor("sidx1", [P, BPC * CAP * 8], i16, kind="ExternalInput")
    di_ext = nc.dram_tensor("didx", [P, 2 * BPC * CAP * 8], i16, kind="ExternalInput")
    dl_ext = nc.dram_tensor("dloct", [P, 2 * BPC * CAP], bf16, kind="ExternalInput")
    beta_ext = nc.dram_tensor("beta_b", [P, N_LAYERS], f32, kind="ExternalInput")
    out_ext = nc.dram_tensor("out", [P, BPC * D], f32, kind="ExternalOutput")

    table1 = nc.dram_tensor("table1", [NPAD, CH], bf16)
    mydst1 = nc.dram_tensor("mydst1", [ROWS_PC, CH], bf16)
    mydst2 = nc.dram_tensor("mydst2", [ROWS_PC, CH], bf16)
    table2 = nc.dram_tensor("table2", [NPAD, CH], bf16, addr_space="Shared")

    RG = [list(range(ncores))]

    with tile.TileContext(nc) as tc, contextlib.ExitStack() as ctx:
        sb = ctx.enter_context(tc.tile_pool(name="sb", bufs=1))
        gpool = ctx.enter_context(tc.tile_pool(name="g", bufs=2))
        one = ctx.enter_context(tc.tile_pool(name="one", bufs=1))
        spool = ctx.enter_context(tc.tile_pool(name="st", bufs=3))
        upool = ctx.enter_context(tc.tile_pool(name="u", bufs=3, space="PSUM"))
        dram = ctx.enter_context(tc.tile_pool(name="dr", bufs=1, space="DRAM"))

        # ---- persistent small tiles ----
        iota_i = sb.tile([P, P], i32)
        nc.gpsimd.iota(iota_i[:], pattern=[[1, P]], base=0, channel_multiplier=0)
        iota_bf = sb.tile([P, P], bf16)
        nc.vector.tensor_copy(iota_bf[:], iota_i[:])
        iota_big = sb.tile([P, P, 2 * TB * CAP], bf16)
        nc.vector.tensor_copy(
            iota_big[:],
            iota_bf[:].unsqueeze(-1).broadcast_to([P, P, 2 * TB * CAP]))
        beta_t = sb.tile([P, N_LAYERS], f32)
        nc.sync.dma_start(beta_t[:], beta_ext[:])

        idx0 = sb.tile([P, BPC * CAP * 8], i16)
        nc.sync.dma_start(idx0[:], s0_ext[:])
        idx1 = sb.tile([P, BPC * CAP * 8], i16)
        nc.sync.dma_start(idx1[:], s1_ext[:])
        idxd = sb.tile([P, 2 * BPC * CAP * 8], i16)
        nc.sync.dma_start(idxd[:], di_ext[:])
        dloct = sb.tile([P, 2 * BPC * CAP], bf16)
        nc.sync.dma_start(dloct[:], dl_ext[:])

        h2acc_t = sb.tile([P, BPC, D], f32, tag="h2acc", name="h2acc")
        h2acc = [h2acc_t, h2acc_t]
        if _GATHERS_ONLY:
            nc.vector.memset(h2acc_t[:], 0)

        NTN = TBN

        def normalize_to_staging(h_ap, nblks, stg):
            """h_ap: [P, nblks, D] f32; stg: [P, nblks(+), CH] bf16 tile."""
            nc.vector.memset(stg[:], 0)
            for o in range(0, nblks, NTN):
                _norm_pass(h_ap[:, o:min(o + NTN, nblks), :],
                           min(NTN, nblks - o),
                           stg[:, o:min(o + NTN, nblks), :])

        def _norm_pass(h_ap, nblks, stg):
            sq = one.tile([P, NTN, D], f32, tag="sq", name="sq")
            nc.scalar.square(sq[:, :nblks, :], h_ap)
            n2 = one.tile([P, NTN], f32, tag="n2")
            nc.vector.tensor_reduce(n2[:, :nblks], sq[:, :nblks, :],
                                    axis=mybir.AxisListType.X,
                                    op=mybir.AluOpType.add)
            nrm = one.tile([P, NTN], f32, tag="nrm")
            nc.scalar.sqrt(nrm[:, :nblks], n2[:, :nblks])
            nc.vector.tensor_scalar_max(nrm[:, :nblks], nrm[:, :nblks], 1e-12)
            inv = one.tile([P, NTN], f32, tag="inv")
            nc.vector.reciprocal(inv[:, :nblks], nrm[:, :nblks])
            nc.vector.tensor_tensor(
                out=stg[:, :nblks, 0:D], in0=h_ap,
                in1=inv[:, :nblks].unsqueeze(-1).broadcast_to([P, nblks, D]),
                op=mybir.AluOpType.mult)
            nc.vector.tensor_copy(stg[:, :nblks, D], nrm[:, :nblks])

        xt_v = xt_ext.ap().rearrange("p (b d) -> p b d", d=D)
        t1_v = table1.ap().rearrange("(b p) c -> p b c", p=P)
        md1_v = mydst1.ap().rearrange("(b p) c -> p b c", p=P)
        md2_v = mydst2.ap().rearrange("(b p) c -> p b c", p=P)

        # ---- layer-1 table: full normalize of x on every core ----
        l1_writes = []
        for i in range(NBLK // TBN):
            xtile = one.tile([P, TBN, D], f32, tag="xt")
            nc.sync.dma_start(xtile[:], xt_v[:, i * TBN:(i + 1) * TBN, :])
            stg = one.tile([P, TBN, CH], bf16, tag="stg")
            normalize_to_staging(xtile[:], TBN, stg)
            l1_writes.append(nc.sync.dma_start(
                t1_v[:, i * TBN:(i + 1) * TBN, :], stg[:]))

        # own-rows (dst) table for layer 1
        rank = nc.gpsimd.cc_rank(RG)
        ownx = one.tile([P, BPC, D], f32, tag="ownx")
        nc.gpsimd.dma_start(out=ownx[:], in_=xt_v[:, bass.ds(rank * BPC, BPC), :])
        stg_own = one.tile([P, BPC, CH], bf16, tag="stgx", name="stg_own")
        normalize_to_staging(ownx[:], BPC, stg_own)
        l1_writes.append(nc.sync.dma_start(md1_v[:], stg_own[:]))

        # ---- per-layer edge pipeline ----
        def emit_layer(layer, src_table_ap, mydst_ap, h2_out, after):
            half0 = src_table_ap[0:H, :]
            half1 = src_table_ap[H:NPAD, :]
            scale_ap = beta_t[:, layer:layer + 1]
            n_tiles = (BPC + TB - 1) // TB
            for t in range(n_tiles):
                blo = t * TB
                tb = min(TB, BPC - blo)
                nsrc = tb * CAP * P
                ndst = 2 * nsrc
                g0 = gpool.tile([P, TB * CAP, CH], bf16, tag="g0")
                gi0 = nc.gpsimd.dma_gather(
                    out_ap=g0[:, 0:tb * CAP, :], in_ap=half0,
                    idxs_ap=idx0[:, blo * CAP * 8:(blo + tb) * CAP * 8],
                    num_idxs=nsrc, num_idxs_reg=nsrc, elem_size=CH,
                    single_packet=False)
                g1 = gpool.tile([P, TB * CAP, CH], bf16, tag="g1")
                gi1 = nc.gpsimd.dma_gather(
                    out_ap=g1[:, 0:tb * CAP, :], in_ap=half1,
                    idxs_ap=idx1[:, blo * CAP * 8:(blo + tb) * CAP * 8],
                    num_idxs=nsrc, num_idxs_reg=nsrc, elem_size=CH,
                    single_packet=False)
                gd = gpool.tile([P, 2 * TB * CAP, CH], bf16, tag="gd")
                gid = nc.gpsimd.dma_gather(
                    out_ap=gd[:, 0:2 * tb * CAP, :], in_ap=mydst_ap,
                    idxs_ap=idxd[:, blo * 2 * CAP * 8:(blo + tb) * 2 * CAP * 8],
                    num_idxs=ndst, num_idxs_reg=ndst, elem_size=CH,
                    single_packet=False)
                for gi in (gi0, gi1, gid):
                    for a in after:
                        bass._add_dep_helper(gi.ins, a.ins, sync=True,
                                             reason="table ready")
                for bi in range(tb):
                    b = blo + bi
                    u = upool.tile([P, 65], mybir.dt.float32, tag="U")
                    for hh in (0, 1):
                        gsrc = (g0, g1)[hh]
                        soff = bi * CAP
                        doff = (2 * bi + hh) * CAP
                        segc = (2 * b + hh) * CAP
                        st = spool.tile([P, CAP, P], bf16, tag="st")
                        nc.vector.tensor_tensor(
                            out=st[:],
                            in0=dloct[:, segc:segc + CAP]
                                .unsqueeze(-1).broadcast_to([P, CAP, P]),
                            in1=iota_bf[:].unsqueeze(1).broadcast_to([P, CAP, P]),
                            op=mybir.AluOpType.is_equal)
                        prod = spool.tile([P, CAP, D], bf16, tag="prod")
                        nc.vector.tensor_tensor(
                            out=prod[:],
                            in0=gsrc[:, soff:soff + CAP, 0:D],
                            in1=gd[:, doff:doff + CAP, 0:D],
                            op=mybir.AluOpType.mult)
                        cos = spool.tile([P, CAP], mybir.dt.float32, tag="cos")
                        nc.vector.tensor_reduce(
                            cos[:], prod[:], axis=mybir.AxisListType.X,
                            op=mybir.AluOpType.add)
                        ee = spool.tile([P, CAP], bf16, tag="ee")
                        nc.scalar.activation(
                            ee[:], cos[:], mybir.ActivationFunctionType.Exp,
                            scale=scale_ap)
                        tt = spool.tile([P, CAP], bf16, tag="tt")
                        nc.vector.tensor_tensor(
                            out=tt[:], in0=ee[:],
                            in1=gsrc[:, soff:soff + CAP, D],
                            op=mybir.AluOpType.mult)
                        w = spool.tile([P, CAP, W_CH], bf16, tag="w")
                        nc.vector.tensor_tensor(
                            out=w[:, :, 0:D],
                            in0=gsrc[:, soff:soff + CAP, 0:D],
                            in1=tt[:].unsqueeze(-1).broadcast_to([P, CAP, D]),
                            op=mybir.AluOpType.mult)
                        nc.vector.tensor_copy(w[:, :, D], ee[:])
                        for k in range(CAP):
                            nc.tensor.matmul(
                                out=u[:],
                                lhsT=st[:, k, :],
                                rhs=w[:, k, 0:D + 1],
                                start=(hh == 0 and k == 0),
                                stop=(hh == 1 and k == CAP - 1))
                    den = spool.tile([P, 1], mybir.dt.float32, tag="den")
                    nc.vector.tensor_scalar_max(den[:], u[:, D:D + 1], 1e-30)
                    dinv = spool.tile([P, 1], mybir.dt.float32, tag="dinv")
                    nc.vector.reciprocal(dinv[:], den[:])
                    nc.vector.tensor_scalar(
                        out=h2_out[:, b, :], in0=u[:, 0:D],
                        scalar1=dinv[:], scalar2=0.0,
                        op0=mybir.AluOpType.mult, op1=mybir.AluOpType.max)

        # ---------------- layer 1 ----------------
        emit_layer(0, table1.ap(), mydst1.ap(), h2acc[0][:], l1_writes)

        # ---------------- boundary: table2 ----------------
        stg_b = one.tile([P, BPC, CH], bf16, tag="stgx", name="stg_b")
        normalize_to_staging(h2acc[0][:], BPC, stg_b)
        md2_w = nc.sync.dma_start(md2_v[:], stg_b[:])
        bounce = dram.tile([ROWS_PC, CH], bf16)
        bounce_v = bounce.rearrange("(b p) c -> p b c", p=P)
        nc.sync.dma_start(bounce_v[:], stg_b[:])
        cc = nc.gpsimd.collective_compute(
            "AllGather", mybir.AluOpType.bypass, replica_groups=RG,
            ins=[bounce.opt()], outs=[table2.ap().opt()])

        # ---------------- layer 2 ----------------
        emit_layer(1, table2.ap(), mydst2.ap(), h2acc[1][:], [cc, md2_w])

        nc.sync.dma_start(
            out_ext.ap().rearrange("p (b d) -> p b d", d=D), h2acc[1][:])

    nc.compile()
    return nc


# ---------------- host wrappers ----------------
def _make_in_maps(x, src, dst, beta):
    per_core = _preprocess(src, dst)
    xpad = np.zeros((NPAD, D), np.float32)
    xpad[:min(N_NODES, NPAD)] = np.asarray(x, np.float32)[:NPAD]
    xt = np.ascontiguousarray(
        xpad.reshape(NBLK, P, D).transpose(1, 0, 2).reshape(P, NBLK * D))
    beta_b = np.repeat(np.asarray(beta, np.float32)[None, :], P, axis=0)
    in_maps = []
    for c in range(N_CORES):
        pc = per_core[c]
        in_maps.append({
            "xt": xt, "beta_b": beta_b,
            "sidx0": pc["sidx0"], "sidx1": pc["sidx1"],
            "didx": pc["didx"], "dloct": pc["dloct"],
        })
    return in_maps


def _unshard_out(results):
    out = np.empty((NPAD, D), np.float32)
    for c in range(N_CORES):
        o = results[c]["out"].reshape(P, BPC, D)
        out[c * ROWS_PC:(c + 1) * ROWS_PC] = \
            o.transpose(1, 0, 2).reshape(ROWS_PC, D)
    return out[:N_NODES]


def kernel(x, src, dst, beta):
    in_maps = _make_in_maps(x, src, dst, beta)
    if "nc" not in _EXEC:
        _EXEC["nc"] = _build()
    res = run_bass_kernel_spmd(_EXEC["nc"], in_maps,
                               core_ids=list(range(N_CORES)))
    return _unshard_out(res.results)


if __name__ == "__main__":
    import reference
    inp = reference.setup_inputs()
    got = kernel(**{k: np.asarray(v) for k, v in inp.items()})
    exp = np.asarray(reference.reference(**inp))
    print("Relative error:", np.linalg.norm(got - exp) / np.linalg.norm(exp))


# revision 11
# speedup vs baseline: 1485.2958x; 1485.2958x over previous
"""AGNN (2-layer) distributed Bass kernel for one TRN2 chip (8 NeuronCores).

Strategy (dst-sharded graph parallel):
  - Nodes padded to NPAD = NBLK*128; core c owns BPC consecutive blocks.
  - Per layer a node table [NPAD, 128ch bf16] lives in DRAM: ch 0..63 =
    l2-normalized features nh, ch 64 = max(||h||, 1e-12), ch 65..127 = 0.
  - Edges sorted by (dst block, src half, src). Each (block, half) segment
    padded to CAP chunks of 128 edges. Src rows fetched with dma_gather
    (ascending idx within a segment => coalesced descriptors); dst rows
    from a private per-core copy of its own rows.
  - Per chunk: cos via row-dot (DVE), ee = exp(beta*cos) (ScalarE), then
    an indicator matmul [U|den] += St.T @ [ee*norm_src*nh_src | ee]
    accumulated in PSUM per block (TensorE).
    Epilogue: h' = relu(U / max(den, tiny)).
  - One AllGather distributes each core's new rows between layers.

kernel(**inputs) takes FULL inputs, returns the FULL [50000, 64] output.
Graph preprocessing (sort/pad/index packing) is host numpy; all feature
compute runs on the 8 NeuronCores in a single NEFF.
"""
import contextlib
import numpy as np
import ml_dtypes

import concourse.bass as bass
import concourse.tile as tile
from concourse import bacc, mybir
from concourse.bass_utils import run_bass_kernel_spmd

BF16 = ml_dtypes.bfloat16

# ---------------- geometry (defaults = the real problem) ----------------
N_NODES = 50000
D = 64
N_LAYERS = 2
N_CORES = 8
P = 128

NBLK = 392                 # node blocks of 128
CAP = 9                    # chunks of 128 edges per (block, half) segment
TB = 3                     # blocks per gather tile
TBN = 28                   # blocks per normalize tile (must divide NBLK)

NPAD = NBLK * P
BPC = NBLK // N_CORES
ROWS_PC = BPC * P
H = NPAD // 2
CH = 128
W_CH = 72

_EXEC = {}
_SKIP = set()      # timing-attribution: 'st','dve','gather','mm','epi'
_GATHERS_ONLY = False
_SINGLE = False    # build single-core (TimelineSim) variant


def _set_geometry(n_nodes, nblk, cap, tb, tbn):
    global N_NODES, NBLK, CAP, TB, TBN, NPAD, BPC, ROWS_PC, H
    N_NODES, NBLK, CAP, TB, TBN = n_nodes, nblk, cap, tb, tbn
    NPAD = NBLK * P
    BPC = NBLK // N_CORES
    ROWS_PC = BPC * P
    H = NPAD // 2
    _EXEC.clear()


# ---------------- host-side graph preprocessing ----------------
def _wrap_idx(idx):
    """int16 [n] -> [128, n//16]: position i -> (partition i%16, col i//16),
    replicated across the 8 Q7 core groups."""
    n = len(idx)
    w = idx.reshape(n // 16, 16).T
    iw = np.empty((P, n // 16), np.int16)
    for rep in range(8):
        iw[rep * 16:(rep + 1) * 16] = w
    return np.ascontiguousarray(iw)


def _preprocess(src, dst):
    src = np.asarray(src, np.int64)
    dst = np.asarray(dst, np.int64)
    blk = dst // P
    half = (src >= H).astype(np.int64)
    order = (np.lexsort((src, half, blk)) if _SORT_BY == 'src'
             else np.lexsort((dst, half, blk)))
    s, d, b, h = src[order], dst[order], blk[order], half[order]

    seg = b * 2 + h
    seg_start = np.searchsorted(seg, np.arange(2 * NBLK))
    seg_end = np.searchsorted(seg, np.arange(2 * NBLK), side="right")

    per_core = []
    ns = BPC * CAP * P
    for c in range(N_CORES):
        b0 = c * BPC
        dbase = c * ROWS_PC
        sidx = [np.zeros(ns, np.int16), np.zeros(ns, np.int16)]
        didx = np.zeros(2 * ns, np.int16)
        dloc = np.full(2 * ns, -1.0, np.float32)
        for bl in range(BPC):
            for hh in (0, 1):
                g = (b0 + bl) * 2 + hh
                lo, hi = seg_start[g], seg_end[g]
                k = hi - lo
                assert k <= CAP * P, f"segment {g}: {k} edges > CAP*128"
                if k == 0:
                    continue
                ss, dd = s[lo:hi], d[lo:hi]
                sb = bl * CAP * P
                db = (bl * 2 + hh) * CAP * P
                v = (ss - hh * H).astype(np.int16)
                sidx[hh][sb:sb + k] = v
                sidx[hh][sb + k:sb + CAP * P] = v[-1]
                vd = (dd - dbase).astype(np.int16)
                didx[db:db + k] = vd
                didx[db + k:db + CAP * P] = vd[-1]
                dloc[db:db + k] = (dd - (b0 + bl) * P).astype(np.float32)
        dloct = np.ascontiguousarray(
            dloc.reshape(2 * BPC * CAP, P).T.astype(BF16))
        per_core.append({
            "sidx0": _wrap_idx(sidx[0]),
            "sidx1": _wrap_idx(sidx[1]),
            "didx": _wrap_idx(didx),
            "dloct": dloct,
        })
    return per_core


# ---------------- device kernel builder ----------------
def _build():
    nc = bacc.Bacc("TRN2", target_bir_lowering=False, debug=False,
                   num_devices=N_CORES)
    f32, bf16, i16, i32 = (mybir.dt.float32, mybir.dt.bfloat16,
                           mybir.dt.int16, mybir.dt.int32)

    xt_ext = nc.dram_tensor("xt", [P, NBLK * D], f32, kind="ExternalInput")
    s0_ext = nc.dram_tensor("sidx0", [P, BPC * CAP * 8], i16, kind="ExternalInput")
    s1_ext = nc.dram_tensor("sidx1", [P, BPC * CAP * 8], i16, kind="ExternalInput")
    di_ext = nc.dram_tensor("didx", [P, 2 * BPC * CAP * 8], i16, kind="ExternalInput")
    dl_ext = nc.dram_tensor("dloct", [P, 2 * BPC * CAP], bf16, kind="ExternalInput")
    beta_ext = nc.dram_tensor("beta_b", [P, N_LAYERS], f32, kind="ExternalInput")
    out_ext = nc.dram_tensor("out", [P, BPC * D], f32, kind="ExternalOutput")

    table1 = nc.dram_tensor("table1", [NPAD, CH], bf16)
    mydst1 = nc.dram_tensor("mydst1", [ROWS_PC, CH], bf16)
    mydst2 = nc.dram_tensor("mydst2", [ROWS_PC, CH], bf16)
    table2 = nc.dram_tensor("table2", [NPAD, CH], bf16, addr_space="Shared")

    RG = [list(range(ncores))]

    with tile.TileContext(nc) as tc, contextlib.ExitStack() as ctx:
        sb = ctx.enter_context(tc.tile_pool(name="sb", bufs=1))
        gpool = ctx.enter_context(tc.tile_pool(name="g", bufs=2))
        one = ctx.enter_context(tc.tile_pool(name="one", bufs=1))
        spool = ctx.enter_context(tc.tile_pool(name="st", bufs=3))
        upool = ctx.enter_context(tc.tile_pool(name="u", bufs=3, space="PSUM"))
        dram = ctx.enter_context(tc.tile_pool(name="dr", bufs=1, space="DRAM"))

        # ---- persistent small tiles ----
        iota_i = sb.tile([P, P], i32)
        nc.gpsimd.iota(iota_i[:], pattern=[[1, P]], base=0, channel_multiplier=0)
        iota_bf = sb.tile([P, P], bf16)
        nc.vector.tensor_copy(iota_bf[:], iota_i[:])
        iota_big = sb.tile([P, P, 2 * TB * CAP], bf16)
        nc.vector.tensor_copy(
            iota_big[:],
            iota_bf[:].unsqueeze(-1).broadcast_to([P, P, 2 * TB * CAP]))
        beta_t = sb.tile([P, N_LAYERS], f32)
        nc.sync.dma_start(beta_t[:], beta_ext[:])

        idx0 = sb.tile([P, BPC * CAP * 8], i16)
        nc.sync.dma_start(idx0[:], s0_ext[:])
        idx1 = sb.tile([P, BPC * CAP * 8], i16)
        nc.sync.dma_start(idx1[:], s1_ext[:])
        idxd = sb.tile([P, 2 * BPC * CAP * 8], i16)
        nc.sync.dma_start(idxd[:], di_ext[:])
        dloct = sb.tile([P, 2 * BPC * CAP], bf16)
        nc.sync.dma_start(dloct[:], dl_ext[:])

        h2acc_t = sb.tile([P, BPC, D], f32, tag="h2acc", name="h2acc")
        h2acc = [h2acc_t, h2acc_t]
        if _GATHERS_ONLY:
            nc.vector.memset(h2acc_t[:], 0)

        NTN = TBN

        def normalize_to_staging(h_ap, nblks, stg):
            """h_ap: [P, nblks, D] f32; stg: [P, nblks(+), CH] bf16 tile."""
            nc.vector.memset(stg[:], 0)
            for o in range(0, nblks, NTN):
                _norm_pass(h_ap[:, o:min(o + NTN, nblks), :],
                           min(NTN, nblks - o),
                           stg[:, o:min(o + NTN, nblks), :])

        def _norm_pass(h_ap, nblks, stg):
            sq = one.tile([P, NTN, D], f32, tag="sq", name="sq")
            nc.scalar.square(sq[:, :nblks, :], h_ap)
            n2 = one.tile([P, NTN], f32, tag="n2")
            nc.vector.tensor_reduce(n2[:, :nblks], sq[:, :nblks, :],
                                    axis=mybir.AxisListType.X,
                                    op=mybir.AluOpType.add)
            nrm = one.tile([P, NTN], f32, tag="nrm")
            nc.scalar.sqrt(nrm[:, :nblks], n2[:, :nblks])
            nc.vector.tensor_scalar_max(nrm[:, :nblks], nrm[:, :nblks], 1e-12)
            inv = one.tile([P, NTN], f32, tag="inv")
            nc.vector.reciprocal(inv[:, :nblks], nrm[:, :nblks])
            nc.vector.tensor_tensor(
                out=stg[:, :nblks, 0:D], in0=h_ap,
                in1=inv[:, :nblks].unsqueeze(-1).broadcast_to([P, nblks, D]),
                op=mybir.AluOpType.mult)
            nc.vector.tensor_copy(stg[:, :nblks, D], nrm[:, :nblks])

        xt_v = xt_ext.ap().rearrange("p (b d) -> p b d", d=D)
        t1_v = table1.ap().rearrange("(b p) c -> p b c", p=P)
        md1_v = mydst1.ap().rearrange("(b p) c -> p b c", p=P)
        md2_v = mydst2.ap().rearrange("(b p) c -> p b c", p=P)

        # ---- layer-1 table: full normalize of x on every core ----
        l1_writes = []
        for i in range(NBLK // TBN):
            xtile = one.tile([P, TBN, D], f32, tag="xt")
            nc.sync.dma_start(xtile[:], xt_v[:, i * TBN:(i + 1) * TBN, :])
            stg = one.tile([P, TBN, CH], bf16, tag="stg")
            normalize_to_staging(xtile[:], TBN, stg)
            l1_writes.append(nc.sync.dma_start(
                t1_v[:, i * TBN:(i + 1) * TBN, :], stg[:]))

        # own-rows (dst) table for layer 1
        rank = nc.gpsimd.cc_rank(RG)
        ownx = one.tile([P, BPC, D], f32, tag="ownx")
        nc.gpsimd.dma_start(out=ownx[:], in_=xt_v[:, bass.ds(rank * BPC, BPC), :])
        stg_own = one.tile([P, BPC, CH], bf16, tag="stgx", name="stg_own")
        normalize_to_staging(ownx[:], BPC, stg_own)
        l1_writes.append(nc.sync.dma_start(md1_v[:], stg_own[:]))

        # ---- per-layer edge pipeline ----
        def emit_layer(layer, src_table_ap, mydst_ap, h2_out, after):
            half0 = src_table_ap[0:H, :]
            half1 = src_table_ap[H:NPAD, :]
            scale_ap = beta_t[:, layer:layer + 1]
            n_tiles = (BPC + TB - 1) // TB
            for t in range(n_tiles):
                blo = t * TB
                tb = min(TB, BPC - blo)
                nsrc = tb * CAP * P
                ndst = 2 * nsrc
                g0 = gpool.tile([P, TB * CAP, CH], bf16, tag="g0")
                gi0 = nc.gpsimd.dma_gather(
                    out_ap=g0[:, 0:tb * CAP, :], in_ap=half0,
                    idxs_ap=idx0[:, blo * CAP * 8:(blo + tb) * CAP * 8],
                    num_idxs=nsrc, num_idxs_reg=nsrc, elem_size=CH,
                    single_packet=False)
                g1 = gpool.tile([P, TB * CAP, CH], bf16, tag="g1")
                gi1 = nc.gpsimd.dma_gather(
                    out_ap=g1[:, 0:tb * CAP, :], in_ap=half1,
                    idxs_ap=idx1[:, blo * CAP * 8:(blo + tb) * CAP * 8],
                    num_idxs=nsrc, num_idxs_reg=nsrc, elem_size=CH,
                    single_packet=False)
                gd = gpool.tile([P, 2 * TB * CAP, CH], bf16, tag="gd")
                gid = nc.gpsimd.dma_gather(
                    out_ap=gd[:, 0:2 * tb * CAP, :], in_ap=mydst_ap,
                    idxs_ap=idxd[:, blo * 2 * CAP * 8:(blo + tb) * 2 * CAP * 8],
                    num_idxs=ndst, num_idxs_reg=ndst, elem_size=CH,
                    single_packet=False)
                for gi in (gi0, gi1, gid):
                    for a in after:
                        bass._add_dep_helper(gi.ins, a.ins, sync=True,
                                             reason="table ready")
                for bi in range(tb):
                    b = blo + bi
                    u = upool.tile([P, 65], mybir.dt.float32, tag="U")
                    for hh in (0, 1):
                        gsrc = (g0, g1)[hh]
                        soff = bi * CAP
                        doff = (2 * bi + hh) * CAP
                        segc = (2 * b + hh) * CAP
                        st = spool.tile([P, CAP, P], bf16, tag="st")
                        nc.vector.tensor_tensor(
                            out=st[:],
                            in0=dloct[:, segc:segc + CAP]
                                .unsqueeze(-1).broadcast_to([P, CAP, P]),
                            in1=iota_bf[:].unsqueeze(1).broadcast_to([P, CAP, P]),
                            op=mybir.AluOpType.is_equal)
                        prod = spool.tile([P, CAP, D], bf16, tag="prod")
                        nc.vector.tensor_tensor(
                            out=prod[:],
                            in0=gsrc[:, soff:soff + CAP, 0:D],
                            in1=gd[:, doff:doff + CAP, 0:D],
                            op=mybir.AluOpType.mult)
                        cos = spool.tile([P, CAP], mybir.dt.float32, tag="cos")
                        nc.vector.tensor_reduce(
                            cos[:], prod[:], axis=mybir.AxisListType.X,
                            op=mybir.AluOpType.add)
                        ee = spool.tile([P, CAP], bf16, tag="ee")
                        nc.scalar.activation(
                            ee[:], cos[:], mybir.ActivationFunctionType.Exp,
                            scale=scale_ap)
                        tt = spool.tile([P, CAP], bf16, tag="tt")
                        nc.vector.tensor_tensor(
                            out=tt[:], in0=ee[:],
                            in1=gsrc[:, soff:soff + CAP, D],
                            op=mybir.AluOpType.mult)
                        w = spool.tile([P, CAP, W_CH], bf16, tag="w")
                        nc.vector.tensor_tensor(
                            out=w[:, :, 0:D],
                            in0=gsrc[:, soff:soff + CAP, 0:D],
                            in1=tt[:].unsqueeze(-1).broadcast_to([P, CAP, D]),
                            op=mybir.AluOpType.mult)
                        nc.vector.tensor_copy(w[:, :, D], ee[:])
                        for k in range(CAP):
                            nc.tensor.matmul(
                                out=u[:],
                                lhsT=st[:, k, :],
                                rhs=w[:, k, 0:D + 1],
                                start=(hh == 0 and k == 0),
                                stop=(hh == 1 and k == CAP - 1))
                    den = spool.tile([P, 1], mybir.dt.float32, tag="den")
                    nc.vector.tensor_scalar_max(den[:], u[:, D:D + 1], 1e-30)
                    dinv = spool.tile([P, 1], mybir.dt.float32, tag="dinv")
                    nc.vector.reciprocal(dinv[:], den[:])
                    nc.vector.tensor_scalar(
                        out=h2_out[:, b, :], in0=u[:, 0:D],
                        scalar1=dinv[:], scalar2=0.0,
                        op0=mybir.AluOpType.mult, op1=mybir.AluOpType.max)

        # ---------------- layer 1 ----------------
        emit_layer(0, table1.ap(), mydst1.ap(), h2acc[0][:], l1_writes)

        # ---------------- boundary: table2 ----------------
        stg_b = one.tile([P, BPC, CH], bf16, tag="stgx", name="stg_b")
        normalize_to_staging(h2acc[0][:], BPC, stg_b)
        md2_w = nc.sync.dma_start(md2_v[:], stg_b[:])
        bounce = dram.tile([ROWS_PC, CH], bf16)
        bounce_v = bounce.rearrange("(b p) c -> p b c", p=P)
        nc.sync.dma_start(bounce_v[:], stg_b[:])
        cc = nc.gpsimd.collective_compute(
            "AllGather", mybir.AluOpType.bypass, replica_groups=RG,
            ins=[bounce.opt()], outs=[table2.ap().opt()])

        # ---------------- layer 2 ----------------
        emit_layer(1, table2.ap(), mydst2.ap(), h2acc[1][:], [cc, md2_w])

        nc.sync.dma_start(
            out_ext.ap().rearrange("p (b d) -> p b d", d=D), h2acc[1][:])

    nc.compile()
    return nc


# ---------------- host wrappers ----------------
def _make_in_maps(x, src, dst, beta):
    per_core = _preprocess(src, dst)
    xpad = np.zeros((NPAD, D), np.float32)
    xpad[:min(N_NODES, NPAD)] = np.asarray(x, np.float32)[:NPAD]
    xt = np.ascontiguousarray(
        xpad.reshape(NBLK, P, D).transpose(1, 0, 2).reshape(P, NBLK * D))
    beta_b = np.repeat(np.asarray(beta, np.float32)[None, :], P, axis=0)
    in_maps = []
    for c in range(N_CORES):
        pc = per_core[c]
        in_maps.append({
            "xt": xt, "beta_b": beta_b,
            "sidx0": pc["sidx0"], "sidx1": pc["sidx1"],
            "didx": pc["didx"], "dloct": pc["dloct"],
        })
    return in_maps


def _unshard_out(results):
    out = np.empty((NPAD, D), np.float32)
    for c in range(N_CORES):
        o = results[c]["out"].reshape(P, BPC, D)
        out[c * ROWS_PC:(c + 1) * ROWS_PC] = \
            o.transpose(1, 0, 2).reshape(ROWS_PC, D)
    return out[:N_NODES]


def kernel(x, src, dst, beta):
    # widen CAP if this graph has a (block, half) segment above the default
    global CAP
    d64 = np.asarray(dst, np.int64)
    s64 = np.asarray(src, np.int64)
    seg = (d64 // P) * 2 + (s64 >= H)
    mx = int(np.bincount(seg, minlength=2 * NBLK).max())
    need = -(-mx // P)
    if need > CAP:
        CAP = need
        _EXEC.clear()
    in_maps = _make_in_maps(x, src, dst, beta)
    if "nc" not in _EXEC:
        _EXEC["nc"] = _build()
    res = run_bass_kernel_spmd(_EXEC["nc"], in_maps,
                               core_ids=list(range(N_CORES)))
    return _unshard_out(res.results)


if __name__ == "__main__":
    import reference
    inp = reference.setup_inputs()
    got = kernel(**{k: np.asarray(v) for k, v in inp.items()})
    exp = np.asarray(reference.reference(**inp))
    print("Relative error:", np.linalg.norm(got - exp) / np.linalg.norm(exp))
